# revision 29
# baseline (speedup 1.0000x reference)
"""Trainium2 Bass kernel for nn_Decoder_36953898615460.

recon[B, D] = einsum('lbf,lfd->bd', acts[:n], W[:n]) + sum(bias[:n], 0)

Strategy (row-parallel over F, 8 NeuronCores), evolved from the 398 us bf16
kernel by moving 1/4 of the contraction to fp8-e4m3 DoubleRow matmuls
(DR-corrected TimelineSim 332 us; measured rel err 1.906e-2 on hw):
  - Shard the contraction dim F across 8 cores: core r owns F columns
    [r*768, (r+1)*768)  ->  local contraction K_loc = n*768 (9216 for n=12).
  - Hybrid precision: the first K8 = 2304 local k-rows (18 k-tiles, q=0.25)
    are fp8-e4m3 on both operands, consumed by DoubleRow matmuls (K=256 per
    instruction; measured 213-230 ns on hw = 2x bf16 MAC rate. The stock
    cost model's 0.5 cycles/row = 4x is wrong on silicon -- hw-validated
    via mm_bench2.py Fori chains -- hence the corrected TimelineSim in
    test.py). The remaining 6912 rows stay bf16. Pure fp8 would be rel err
    3.75e-2; err scales as 3.75e-2*sqrt(q) and PE time as 369us*(1-q/2),
    so q=0.25 (err 1.91e-2 vs the 2e-2 gate, deterministic: quantization
    is host-side and device arithmetic is exact) is the accuracy-limited
    sweet spot. PE floor (54+9) tiles * 6 subtiles * 4 blocks * 213ns
    = 322 us vs 369 us pure bf16.
  - Scale trick: host multiplies ALL of W by 64 (exact exponent shift in
    bf16; lifts fp8 W out of subnormals) and bias by 64; the host divides
    the gathered output by 64. One uniform PSUM scale -> fp8 and bf16
    chunks accumulate into the same PSUM bank with zero extra device ops.
  - Chunk plan (see _plan): one 6-tile fp8 chunk first (smallest first-DMA
    wait), nine 6-tile bf16 chunks, the two remaining fp8 chunks last so
    their 2x-rate matmuls leave the DMA stream maximal slack early in each
    block. W stays SBUF-resident (10.6 MB bf16 + 1.7 MB fp8): streamed once
    during block 0, interleaved per chunk with acts on the SP HWDGE queue;
    early-position W rides the ACT queue (W_ACT) and the first SPLIT_CHUNKS
    positions use sub-chunk DMAs so the first matmuls wait on 2 k-tiles.
  - PE warm-up: dummy N=128 matmuls on zeroed scratch bridge the initial DMA
    wait so the clock ramp burns before the real stream starts.
  - B processed in 4 blocks of 512 (one PSUM bank per d-subtile; full-K
    accumulation per bank, start/stop flags, no SBUF accumulator).
  - Blocks 0..2: per-block bf16-wire ReduceScatter(add) fully overlapped by
    the next block's compute. The LAST block skips the collective (its
    ~17.5 us launch-dominated RS would be fully exposed at the tail): each
    core writes its full [D, 512] bf16 partial to y2 and the host does that
    block's 8-way add during unsharding (also skipping one wire rounding).
    Each block's last chunk runs m-outer with the evacuation issued right
    after that subtile's stop-matmul, on alternating engines, with O_BUFS=6
    output buffers so no evac WAR-waits an earlier DRAM write.
  - bias: each core adds 64*sum_l(bias)/8 during PSUM->SBUF evacuation so
    the 8-way reduce sums to +64*bias (host divides by 64).
  - Output y is block-major bf16 [NBLK-1, 96, 512] + y2 [768, 512]; host
    reassembles the 8 shards, divides by 64, casts to fp32.
"""

import numpy as np
import ml_dtypes

import concourse.mybir as mybir
import concourse.tile as tile
from concourse import bacc
from concourse.bass import ts
from concourse.bass_utils import run_bass_kernel_spmd

NCORES = 8
B, F, D = 2048, 6144, 768
F_LOC = F // NCORES  # 768
P = 128
MD = D // P          # 6 d-subtiles
DR = D // NCORES     # 96 rows per rank after ReduceScatter
BN = 512             # B block width (= matmul moving free dim, one PSUM bank)
NBLK = B // BN       # 4
CK = 8               # preferred k-tiles (of 128) per DMA chunk
FP8_FRAC = 2.0 / 9.0 # target fraction of k-tiles in fp8 (chunk-granular)
WSCALE = 64.0        # host premultiplies W and bias; host divides output
WARM_MMS = 30        # dummy matmuls bridging the initial DMA wait
CK0_SPLITS_A = (2, 4)  # early-chunk acts sub-chunk boundaries (k-tiles)
CK0_SPLITS_W = (2, 4)  # early-chunk W sub-chunk boundaries (k-tiles)
W_ACT = 2            # chunk positions [0..W_ACT] send W down the ACT queue
SPLIT_CHUNKS = 4     # how many leading chunk positions use sub-chunk DMAs
MEMSET_POOL = True   # warmup scratch memset on Pool (True) or DVE (False)
EVAC_SPLIT = False   # split the very last evacuation into two halves
A_BUFS = 4           # in-flight acts chunk buffers (WAR depth on the SP queue)
O_BUFS = 6           # evac output buffers (6 -> no WAR wait across subtiles)
BF_NT = 6            # bf16 chunk size in k-tiles (54 must divide evenly)

_nc_cache = {}
last_result = None  # BassKernelResults of the most recent run (for test harness)


def _plan(n_layers: int):
    """Chunk plan, already in PROCESSING ORDER.

    Returns (K_LOC, K8, plan) where plan is a list of (kind, off, nt):
    kind 'f8'|'bf', off = k-tile offset within that region's dram tensor,
    nt = k-tiles in the chunk. The first chunk is fp8 (half the bytes of a
    bf16 chunk -> shortest first-DMA wait); the remaining fp8 chunks go last
    (their 2x-rate matmuls leave the DMA stream maximal slack early in each
    block).
    """
    K_LOC = n_layers * F_LOC
    KT = K_LOC // P
    if n_layers == 12:
        # 18 fp8 tiles (q=0.25, rel err ~1.9e-2) in 3 chunks of 6,
        # 54 bf16 tiles in chunks of BF_NT
        plan = [("f8", 0, 6)]
        plan += [("bf", BF_NT * i, BF_NT) for i in range(54 // BF_NT)]
        plan += [("f8", 6, 6), ("f8", 12, 6)]
        K8 = 18 * P
    else:
        # generic fallback: pure bf16 in chunks of <=CK (correctness path;
        # the harness always uses n=12)
        ck = max(c for c in (CK, 6, 4, 3, 2, 1) if KT % c == 0)
        plan = [("bf", ck * i, ck) for i in range(KT // ck)]
        K8 = 0
    return K_LOC, K8, plan


def _build(n_layers: int):
    K_LOC, K8, plan = _plan(n_layers)
    NCH = len(plan)
    KB = K_LOC - K8                   # bf16 k-rows
    n_f8 = sum(1 for kind, _, _ in plan if kind == "f8")
    max_f8_nt = max([nt for kind, _, nt in plan if kind == "f8"], default=0)
    max_bf_nt = max([nt for kind, _, nt in plan if kind == "bf"], default=0)

    nc = bacc.Bacc(None, num_devices=NCORES)
    if K8:
        a8_ext = nc.dram_tensor("a8", [K8, B], mybir.dt.float8e4, kind="ExternalInput")
        w8_ext = nc.dram_tensor("w8", [K8, D], mybir.dt.float8e4, kind="ExternalInput")
        a8_v = a8_ext[:, :].rearrange("(ko p) b -> p ko b", p=P)  # [128, K8T, B]
        w8_v = w8_ext[:, :].rearrange("(ko p) d -> p ko d", p=P)  # [128, K8T, D]
    a_ext = nc.dram_tensor("a_t", [KB, B], mybir.dt.bfloat16, kind="ExternalInput")
    w_ext = nc.dram_tensor("w", [KB, D], mybir.dt.bfloat16, kind="ExternalInput")
    b_ext = nc.dram_tensor("bias_t", [D, n_layers], mybir.dt.float32, kind="ExternalInput")
    # Blocks 0..NBLK-2 are reduced on-device (per-block ReduceScatter that
    # overlaps the next block's compute) into block-major bf16 y. The LAST
    # block skips the collective entirely -- its RS (~17.5 us, launch-constant
    # dominated) would be fully exposed at the tail -- and instead each core
    # writes its full [D, BN] bf16 partial to y2; the host does that block's
    # 8-way add during unsharding (fp32, so it also skips one wire rounding).
    y_ext = nc.dram_tensor("y", [NBLK - 1, DR, BN], mybir.dt.bfloat16, kind="ExternalOutput")
    y2_ext = nc.dram_tensor("y2", [D, BN], mybir.dt.bfloat16, kind="ExternalOutput")

    # bf16 wire format for the reduce: halves RS payload + partial DMAs.
    partials = [nc.dram_tensor(f"partial{b}", [D, BN], mybir.dt.bfloat16) for b in range(NBLK - 1)]
    reduceds = [nc.dram_tensor(f"reduced{b}", [DR, BN], mybir.dt.bfloat16) for b in range(NBLK - 1)]

    a_v = a_ext[:, :].rearrange("(ko p) b -> p ko b", p=P)  # [128, KBT, B]
    w_v = w_ext[:, :].rearrange("(ko p) d -> p ko d", p=P)  # [128, KBT, D]
    b_v = b_ext[:, :].rearrange("(mo p) l -> p mo l", p=P)  # [128, MD, n]

    DRMODE = mybir.MatmulPerfMode.DoubleRow

    with tile.TileContext(nc) as tc:
        with (
            tc.tile_pool(name="apool", bufs=A_BUFS) as apool,
            tc.tile_pool(name="a8pool", bufs=3) as a8pool,
            tc.tile_pool(name="wpool", bufs=max(NCH - n_f8, 1)) as wpool,
            tc.tile_pool(name="w8pool", bufs=max(n_f8, 1)) as w8pool,
            tc.tile_pool(name="cpool", bufs=1) as cpool,
            tc.tile_pool(name="opool", bufs=O_BUFS) as opool,
            tc.tile_pool(name="pspool", bufs=8, space="PSUM") as pspool,
        ):
            # bias tiles allocated up front; the DMA is emitted later (inside
            # block 0's chunk loop) so its transfer can't slip into the
            # critical first-W window on the serialized DMA device
            bias_t = cpool.tile([P, MD, n_layers], mybir.dt.float32)
            bias8 = cpool.tile([P, MD], mybir.dt.float32)

            def load_bias():
                nc.sync.dma_start(bias_t[:], b_v)
                nc.vector.reduce_sum(bias8[:], bias_t[:], axis=mybir.AxisListType.X)
                nc.vector.tensor_scalar_mul(bias8[:], bias8[:], 1.0 / NCORES)

            # PE warm-up: a dense dummy-matmul chain that spans the first
            # acts/W DMA wait so the clock ramp (half-rate for the first ~3us
            # of PE activity) is burned before the real stream starts.
            # memset on Pool: its SEQ is free at t=0, so the PE's first
            # Ldweights waits ~0.25us instead of ~1us behind the DVE queue
            scratch = cpool.tile([P, P], mybir.dt.bfloat16)
            (nc.gpsimd if MEMSET_POOL else nc.vector).memset(scratch[:], 0)
            ps_warm = pspool.tile([P, P], mybir.dt.float32, tag="ps", name="ps_warm")
            for i in range(WARM_MMS):
                nc.tensor.matmul(
                    ps_warm[:], scratch[:], scratch[:],
                    start=(i == 0), stop=(i == WARM_MMS - 1),
                )

            def evac(blk, m, ps_m):
                """PSUM -> SBUF (+bias) -> DRAM, alternating engines."""
                ob = opool.tile([P, BN], mybir.dt.bfloat16, tag="o", name=f"ob{blk}_{m}")
                if EVAC_SPLIT and blk == NBLK - 1 and m == MD - 1:
                    # very last evac: split into halves on separate engines
                    # AND separate HWDGE queues so the two write dispatches
                    # overlap -- trims the exposed tail
                    h = BN // 2
                    nc.scalar.add(ob[:, :h], ps_m[:, :h], bias8[:, m : m + 1])
                    nc.scalar.dma_start(y2_ext[ts(m, P), 0:h], ob[:, :h])
                    nc.vector.tensor_scalar_add(ob[:, h:], ps_m[:, h:], bias8[:, m : m + 1])
                    nc.sync.dma_start(y2_ext[ts(m, P), h:BN], ob[:, h:])
                    return
                if m in (0, 2, 5):  # m5 on DVE: the ACT queue then pre-stages
                    nc.vector.tensor_scalar_add(ob[:], ps_m[:], bias8[:, m : m + 1])
                else:               # its output-DMA dispatch during the evac
                    nc.scalar.add(ob[:], ps_m[:], bias8[:, m : m + 1])
                if blk < NBLK - 1:
                    nc.scalar.dma_start(partials[blk][ts(m, P), :], ob[:])
                else:
                    # alternate queues so the six y2 write dispatches don't
                    # serialize on one HWDGE; the last lands on the idle SP
                    q = nc.sync if m % 2 else nc.scalar
                    q.dma_start(y2_ext[ts(m, P), :], ob[:])

            w_tile_by_pos = {}
            for blk in range(NBLK):
                b0 = blk * BN
                ps = [pspool.tile([P, BN], mybir.dt.float32, tag="ps", name=f"ps{blk}_{m}") for m in range(MD)]
                for pos, (kind, off, nt) in enumerate(plan):
                    is8 = kind == "f8"
                    last = pos == NCH - 1
                    if is8:
                        a_c = a8pool.tile([P, max_f8_nt, BN], mybir.dt.float8e4, tag="a8")
                        src_a = a8_v[:, off : off + nt, b0 : b0 + BN]
                    else:
                        a_c = apool.tile([P, max_bf_nt, BN], mybir.dt.bfloat16, tag="a")
                        src_a = a_v[:, off : off + nt, b0 : b0 + BN]
                    if blk == 0:
                        if is8:
                            w_c = w8pool.tile([P, max_f8_nt, D], mybir.dt.float8e4, tag="w8")
                            src_w = w8_v[:, off : off + nt, :]
                        else:
                            w_c = wpool.tile([P, max_bf_nt, D], mybir.dt.bfloat16, tag="w")
                            src_w = w_v[:, off : off + nt, :]
                        w_tile_by_pos[pos] = w_c
                        if pos < SPLIT_CHUNKS:
                            # early chunks in sub-chunk DMAs: matmuls wait on
                            # a fraction of a chunk while per-DMA dispatch
                            # overhead stays amortized. Chunk-0 W rides the
                            # ACT queue so both HWDGE queues ramp in parallel.
                            wq = nc.scalar if pos <= W_ACT else nc.sync
                            prev = 0
                            for h1 in [x for x in CK0_SPLITS_A if x < nt] + [nt]:
                                nc.sync.dma_start(a_c[:, prev:h1], src_a[:, prev:h1])
                                prev = h1
                            prev = 0
                            for h1 in [x for x in CK0_SPLITS_W if x < nt] + [nt]:
                                wq.dma_start(w_c[:, prev:h1], src_w[:, prev:h1])
                                prev = h1
                        else:
                            nc.sync.dma_start(a_c[:, :nt], src_a)
                            if pos <= W_ACT:
                                nc.scalar.dma_start(w_c[:, :nt], src_w)
                            else:
                                nc.sync.dma_start(w_c[:, :nt], src_w)
                    else:
                        nc.sync.dma_start(a_c[:, :nt], src_a)
                        w_c = w_tile_by_pos[pos]
                    if blk == 0 and pos == SPLIT_CHUNKS:
                        load_bias()
                    if is8:
                        # DoubleRow: one matmul per k-tile PAIR (K=256).
                        # On the last chunk run m-outer so each subtile's
                        # accumulation finishes early and its evacuation
                        # overlaps the rest.
                        if last:
                            for m in range(MD):
                                for j in range(nt // 2):
                                    nc.tensor.matmul(
                                        ps[m][:],
                                        w_c[:, 2 * j : 2 * j + 2, ts(m, P)],
                                        a_c[:, 2 * j : 2 * j + 2],
                                        start=(pos == 0 and j == 0),
                                        stop=(j == nt // 2 - 1),
                                        perf_mode=DRMODE,
                                    )
                                evac(blk, m, ps[m])
                        else:
                            for j in range(nt // 2):
                                for m in range(MD):
                                    nc.tensor.matmul(
                                        ps[m][:],
                                        w_c[:, 2 * j : 2 * j + 2, ts(m, P)],
                                        a_c[:, 2 * j : 2 * j + 2],
                                        start=(pos == 0 and j == 0),
                                        stop=False,
                                        perf_mode=DRMODE,
                                    )
                    elif not last:
                        for k in range(nt):
                            for m in range(MD):
                                nc.tensor.matmul(
                                    ps[m][:],
                                    w_c[:, k, ts(m, P)],
                                    a_c[:, k],
                                    start=(pos == 0 and k == 0),
                                    stop=False,
                                )
                    else:
                        for m in range(MD):
                            for k in range(nt):
                                nc.tensor.matmul(
                                    ps[m][:],
                                    w_c[:, k, ts(m, P)],
                                    a_c[:, k],
                                    start=(pos == 0 and k == 0),
                                    stop=(k == nt - 1),
                                )
                            evac(blk, m, ps[m])

                if blk < NBLK - 1:
                    nc.gpsimd.collective_compute(
                        "ReduceScatter",
                        mybir.AluOpType.add,
                        replica_groups=[list(range(NCORES))],
                        ins=[partials[blk][:, :].opt()],
                        outs=[reduceds[blk][:, :].opt()],
                    )
                    nc.gpsimd.dma_start(y_ext[blk], reduceds[blk][:, :])
    nc.compile()
    return nc


def _get_nc(n_layers: int):
    if n_layers not in _nc_cache:
        _nc_cache[n_layers] = _build(n_layers)
    return _nc_cache[n_layers]


def kernel(acts: np.ndarray, W: np.ndarray, bias: np.ndarray, layer_idx) -> np.ndarray:
    global last_result
    n = int(layer_idx) + 1
    bf16 = ml_dtypes.bfloat16
    f8 = ml_dtypes.float8_e4m3
    K_LOC, K8, _ = _plan(n)

    acts32 = np.asarray(acts, dtype=np.float32)[:n]          # [n, B, F]
    W64 = np.asarray(W, dtype=np.float32)[:n] * WSCALE       # [n, F, D]
    bias = np.asarray(bias, dtype=np.float32)[:n]            # [n, D]

    nc = _get_nc(n)

    bias_t = np.ascontiguousarray(bias.T) * WSCALE  # [D, n], same on every core
    in_maps = []
    for r in range(NCORES):
        f0 = r * F_LOC
        # [n, B, F_LOC] -> [n, F_LOC, B] -> [K_loc, B]
        a_loc = np.ascontiguousarray(
            acts32[:, :, f0 : f0 + F_LOC].transpose(0, 2, 1)
        ).reshape(K_LOC, B)
        w_loc = np.ascontiguousarray(W64[:, f0 : f0 + F_LOC, :]).reshape(K_LOC, D)
        m = {
            "a_t": a_loc[K8:].astype(bf16),
            "w": w_loc[K8:].astype(bf16),
            "bias_t": bias_t,
        }
        if K8:
            m["a8"] = a_loc[:K8].astype(f8)
            m["w8"] = w_loc[:K8].astype(f8)
        in_maps.append(m)

    def run_once():
        global last_result
        last_result = run_bass_kernel_spmd(nc, in_maps, core_ids=list(range(NCORES)))
        out = np.empty((D, B), dtype=np.float32)
        last = np.zeros((D, BN), dtype=np.float32)
        for r in range(NCORES):
            y_r = np.asarray(last_result.results[r]["y"]).astype(np.float32)  # [NBLK-1, DR, BN]
            for blk in range(NBLK - 1):
                out[r * DR : (r + 1) * DR, blk * BN : (blk + 1) * BN] = y_r[blk]
            # last block: device skipped the collective; 8-way add here
            last += np.asarray(last_result.results[r]["y2"]).astype(np.float32)
        out[:, (NBLK - 1) * BN :] = last
        out *= 1.0 / WSCALE
        return np.ascontiguousarray(out.T)  # [B, D] float32

    # Guard against transient device flakes (observed: one core's contribution
    # missing from one block; first-run-after-compile stale outputs): spot-check
    # a few output elements per block against a host fp32 dot product and retry
    # the device run once on gross mismatch.
    # Threshold 0.75 clears the hybrid's quantization error (per-element err
    # std ~0.10) while still flagging a missing core-block (|diff| ~ 1.9).
    rng = np.random.default_rng(0)
    checks = [
        (int(rng.integers(blk * BN, (blk + 1) * BN)), int(m * P + rng.integers(0, P)))
        for blk in range(NBLK)
        for m in range(MD)
        for _ in range(2)
    ]
    W32 = np.asarray(W, dtype=np.float32)[:n]
    bias_sum = bias.sum(axis=0)

    def looks_good(out):
        for b_i, d_i in checks:
            ref = float(np.dot(acts32[:, b_i, :].ravel(), W32[:, :, d_i].ravel())) + float(bias_sum[d_i])
            if abs(out[b_i, d_i] - ref) > 0.75:
                return False
        return True

    out = run_once()
    if not looks_good(out):
        out = run_once()
    return out


# revision 32
# speedup vs baseline: 1.0022x; 1.0022x over previous
"""Trainium2 Bass kernel for nn_Decoder_36953898615460.

recon[B, D] = einsum('lbf,lfd->bd', acts[:n], W[:n]) + sum(bias[:n], 0)

Strategy (row-parallel over F, 8 NeuronCores), evolved from the 398 us bf16
kernel by moving 1/4 of the contraction to fp8-e4m3 DoubleRow matmuls
(DR-corrected TimelineSim 332 us; measured rel err 1.906e-2 on hw):
  - Shard the contraction dim F across 8 cores: core r owns F columns
    [r*768, (r+1)*768)  ->  local contraction K_loc = n*768 (9216 for n=12).
  - Hybrid precision: the first K8 = 2304 local k-rows (18 k-tiles, q=0.25)
    are fp8-e4m3 on both operands, consumed by DoubleRow matmuls (K=256 per
    instruction; measured 213-230 ns on hw = 2x bf16 MAC rate. The stock
    cost model's 0.5 cycles/row = 4x is wrong on silicon -- hw-validated
    via mm_bench2.py Fori chains -- hence the corrected TimelineSim in
    test.py). The remaining 6912 rows stay bf16. Pure fp8 would be rel err
    3.75e-2; err scales as 3.75e-2*sqrt(q) and PE time as 369us*(1-q/2),
    so q=0.25 (err 1.91e-2 vs the 2e-2 gate, deterministic: quantization
    is host-side and device arithmetic is exact) is the accuracy-limited
    sweet spot. PE floor (54+9) tiles * 6 subtiles * 4 blocks * 213ns
    = 322 us vs 369 us pure bf16.
  - Scale trick: host multiplies ALL of W by 64 (exact exponent shift in
    bf16; lifts fp8 W out of subnormals) and bias by 64; the host divides
    the gathered output by 64. One uniform PSUM scale -> fp8 and bf16
    chunks accumulate into the same PSUM bank with zero extra device ops.
  - Chunk plan (see _plan): one 6-tile fp8 chunk first (smallest first-DMA
    wait), nine 6-tile bf16 chunks, the two remaining fp8 chunks last so
    their 2x-rate matmuls leave the DMA stream maximal slack early in each
    block. W stays SBUF-resident (10.6 MB bf16 + 1.7 MB fp8): streamed once
    during block 0, interleaved per chunk with acts on the SP HWDGE queue;
    early-position W rides the ACT queue (W_ACT) and the first SPLIT_CHUNKS
    positions use sub-chunk DMAs so the first matmuls wait on 2 k-tiles.
  - PE warm-up: dummy N=128 matmuls on zeroed scratch bridge the initial DMA
    wait so the clock ramp burns before the real stream starts.
  - B processed in 4 blocks of 512 (one PSUM bank per d-subtile; full-K
    accumulation per bank, start/stop flags, no SBUF accumulator).
  - Blocks 0..2: per-block bf16-wire ReduceScatter(add) fully overlapped by
    the next block's compute. The LAST block skips the collective (its
    ~17.5 us launch-dominated RS would be fully exposed at the tail): each
    core writes its full [D, 512] bf16 partial to y2 and the host does that
    block's 8-way add during unsharding (also skipping one wire rounding).
    Each block's last chunk runs m-outer with the evacuation issued right
    after that subtile's stop-matmul, on alternating engines, with O_BUFS=6
    output buffers so no evac WAR-waits an earlier DRAM write.
  - bias: each core adds 64*sum_l(bias)/8 during PSUM->SBUF evacuation so
    the 8-way reduce sums to +64*bias (host divides by 64).
  - Output y is block-major bf16 [NBLK-1, 96, 512] + y2 [768, 512]; host
    reassembles the 8 shards, divides by 64, casts to fp32.
"""

import numpy as np
import ml_dtypes

import concourse.mybir as mybir
import concourse.tile as tile
from concourse import bacc
from concourse.bass import ts
from concourse.bass_utils import run_bass_kernel_spmd

NCORES = 8
B, F, D = 2048, 6144, 768
F_LOC = F // NCORES  # 768
P = 128
MD = D // P          # 6 d-subtiles
DR = D // NCORES     # 96 rows per rank after ReduceScatter
BN = 512             # B block width (= matmul moving free dim, one PSUM bank)
NBLK = B // BN       # 4
CK = 8               # preferred k-tiles (of 128) per DMA chunk
FP8_FRAC = 2.0 / 9.0 # target fraction of k-tiles in fp8 (chunk-granular)
WSCALE = 64.0        # host premultiplies W and bias; host divides output
WARM_MMS = 30        # dummy matmuls bridging the initial DMA wait
CK0_SPLITS_A = (2, 4)  # early-chunk acts sub-chunk boundaries (k-tiles)
CK0_SPLITS_W = (2, 4)  # early-chunk W sub-chunk boundaries (k-tiles)
W_ACT = 2            # chunk positions [0..W_ACT] send W down the ACT queue
SPLIT_CHUNKS = 4     # how many leading chunk positions use sub-chunk DMAs
MEMSET_POOL = True   # warmup scratch memset on Pool (True) or DVE (False)
EVAC_SPLIT = False   # split the very last evacuation into two halves
A_BUFS = 4           # in-flight acts chunk buffers (WAR depth on the SP queue)
O_BUFS = 6           # evac output buffers (6 -> no WAR wait across subtiles)
BF_NT = 6            # bf16 chunk size in k-tiles (54 must divide evenly)
F8_SPLIT = (4, 6, 8) # fp8 chunk sizes in k-tiles (sum 18, each even)

_nc_cache = {}
last_result = None  # BassKernelResults of the most recent run (for test harness)


def _plan(n_layers: int):
    """Chunk plan, already in PROCESSING ORDER.

    Returns (K_LOC, K8, plan) where plan is a list of (kind, off, nt):
    kind 'f8'|'bf', off = k-tile offset within that region's dram tensor,
    nt = k-tiles in the chunk. The first chunk is fp8 (half the bytes of a
    bf16 chunk -> shortest first-DMA wait); the remaining fp8 chunks go last
    (their 2x-rate matmuls leave the DMA stream maximal slack early in each
    block).
    """
    K_LOC = n_layers * F_LOC
    KT = K_LOC // P
    if n_layers == 12:
        # 18 fp8 tiles (q=0.25, rel err ~1.9e-2) split per F8_SPLIT,
        # 54 bf16 tiles in chunks of BF_NT
        f8_nts = list(F8_SPLIT)
        assert sum(f8_nts) == 18 and all(nt % 2 == 0 for nt in f8_nts)
        offs = [sum(f8_nts[:i]) for i in range(len(f8_nts))]
        plan = [("f8", offs[0], f8_nts[0])]
        plan += [("bf", BF_NT * i, BF_NT) for i in range(54 // BF_NT)]
        plan += [("f8", o, nt) for o, nt in zip(offs[1:], f8_nts[1:])]
        K8 = 18 * P
    else:
        # generic fallback: pure bf16 in chunks of <=CK (correctness path;
        # the harness always uses n=12)
        ck = max(c for c in (CK, 6, 4, 3, 2, 1) if KT % c == 0)
        plan = [("bf", ck * i, ck) for i in range(KT // ck)]
        K8 = 0
    return K_LOC, K8, plan


def _build(n_layers: int):
    K_LOC, K8, plan = _plan(n_layers)
    NCH = len(plan)
    KB = K_LOC - K8                   # bf16 k-rows
    n_f8 = sum(1 for kind, _, _ in plan if kind == "f8")
    max_f8_nt = max([nt for kind, _, nt in plan if kind == "f8"], default=0)
    max_bf_nt = max([nt for kind, _, nt in plan if kind == "bf"], default=0)

    nc = bacc.Bacc(None, num_devices=NCORES)
    if K8:
        a8_ext = nc.dram_tensor("a8", [K8, B], mybir.dt.float8e4, kind="ExternalInput")
        w8_ext = nc.dram_tensor("w8", [K8, D], mybir.dt.float8e4, kind="ExternalInput")
        a8_v = a8_ext[:, :].rearrange("(ko p) b -> p ko b", p=P)  # [128, K8T, B]
        w8_v = w8_ext[:, :].rearrange("(ko p) d -> p ko d", p=P)  # [128, K8T, D]
    a_ext = nc.dram_tensor("a_t", [KB, B], mybir.dt.bfloat16, kind="ExternalInput")
    w_ext = nc.dram_tensor("w", [KB, D], mybir.dt.bfloat16, kind="ExternalInput")
    b_ext = nc.dram_tensor("bias_t", [D, n_layers], mybir.dt.float32, kind="ExternalInput")
    # Blocks 0..NBLK-2 are reduced on-device (per-block ReduceScatter that
    # overlaps the next block's compute) into block-major bf16 y. The LAST
    # block skips the collective entirely -- its RS (~17.5 us, launch-constant
    # dominated) would be fully exposed at the tail -- and instead each core
    # writes its full [D, BN] bf16 partial to y2; the host does that block's
    # 8-way add during unsharding (fp32, so it also skips one wire rounding).
    y_ext = nc.dram_tensor("y", [NBLK - 1, DR, BN], mybir.dt.bfloat16, kind="ExternalOutput")
    y2_ext = nc.dram_tensor("y2", [D, BN], mybir.dt.bfloat16, kind="ExternalOutput")

    # bf16 wire format for the reduce: halves RS payload + partial DMAs.
    partials = [nc.dram_tensor(f"partial{b}", [D, BN], mybir.dt.bfloat16) for b in range(NBLK - 1)]
    reduceds = [nc.dram_tensor(f"reduced{b}", [DR, BN], mybir.dt.bfloat16) for b in range(NBLK - 1)]

    a_v = a_ext[:, :].rearrange("(ko p) b -> p ko b", p=P)  # [128, KBT, B]
    w_v = w_ext[:, :].rearrange("(ko p) d -> p ko d", p=P)  # [128, KBT, D]
    b_v = b_ext[:, :].rearrange("(mo p) l -> p mo l", p=P)  # [128, MD, n]

    DRMODE = mybir.MatmulPerfMode.DoubleRow

    with tile.TileContext(nc) as tc:
        with (
            tc.tile_pool(name="apool", bufs=A_BUFS) as apool,
            tc.tile_pool(name="a8pool", bufs=3) as a8pool,
            tc.tile_pool(name="wpool", bufs=max(NCH - n_f8, 1)) as wpool,
            tc.tile_pool(name="w8pool", bufs=max(n_f8, 1)) as w8pool,
            tc.tile_pool(name="cpool", bufs=1) as cpool,
            tc.tile_pool(name="opool", bufs=O_BUFS) as opool,
            tc.tile_pool(name="pspool", bufs=8, space="PSUM") as pspool,
        ):
            # bias tiles allocated up front; the DMA is emitted later (inside
            # block 0's chunk loop) so its transfer can't slip into the
            # critical first-W window on the serialized DMA device
            bias_t = cpool.tile([P, MD, n_layers], mybir.dt.float32)
            bias8 = cpool.tile([P, MD], mybir.dt.float32)

            def load_bias():
                nc.sync.dma_start(bias_t[:], b_v)
                nc.vector.reduce_sum(bias8[:], bias_t[:], axis=mybir.AxisListType.X)
                nc.vector.tensor_scalar_mul(bias8[:], bias8[:], 1.0 / NCORES)

            # PE warm-up: a dense dummy-matmul chain that spans the first
            # acts/W DMA wait so the clock ramp (half-rate for the first ~3us
            # of PE activity) is burned before the real stream starts.
            # memset on Pool: its SEQ is free at t=0, so the PE's first
            # Ldweights waits ~0.25us instead of ~1us behind the DVE queue
            scratch = cpool.tile([P, P], mybir.dt.bfloat16)
            (nc.gpsimd if MEMSET_POOL else nc.vector).memset(scratch[:], 0)
            ps_warm = pspool.tile([P, P], mybir.dt.float32, tag="ps", name="ps_warm")
            for i in range(WARM_MMS):
                nc.tensor.matmul(
                    ps_warm[:], scratch[:], scratch[:],
                    start=(i == 0), stop=(i == WARM_MMS - 1),
                )

            def evac(blk, m, ps_m):
                """PSUM -> SBUF (+bias) -> DRAM, alternating engines."""
                ob = opool.tile([P, BN], mybir.dt.bfloat16, tag="o", name=f"ob{blk}_{m}")
                if EVAC_SPLIT and blk == NBLK - 1 and m == MD - 1:
                    # very last evac: split into halves on separate engines
                    # AND separate HWDGE queues so the two write dispatches
                    # overlap -- trims the exposed tail
                    h = BN // 2
                    nc.scalar.add(ob[:, :h], ps_m[:, :h], bias8[:, m : m + 1])
                    nc.scalar.dma_start(y2_ext[ts(m, P), 0:h], ob[:, :h])
                    nc.vector.tensor_scalar_add(ob[:, h:], ps_m[:, h:], bias8[:, m : m + 1])
                    nc.sync.dma_start(y2_ext[ts(m, P), h:BN], ob[:, h:])
                    return
                if m in (0, 2, 5):  # m5 on DVE: the ACT queue then pre-stages
                    nc.vector.tensor_scalar_add(ob[:], ps_m[:], bias8[:, m : m + 1])
                else:               # its output-DMA dispatch during the evac
                    nc.scalar.add(ob[:], ps_m[:], bias8[:, m : m + 1])
                if blk < NBLK - 1:
                    nc.scalar.dma_start(partials[blk][ts(m, P), :], ob[:])
                else:
                    # alternate queues so the six y2 write dispatches don't
                    # serialize on one HWDGE; the last lands on the idle SP
                    q = nc.sync if m % 2 else nc.scalar
                    q.dma_start(y2_ext[ts(m, P), :], ob[:])

            w_tile_by_pos = {}
            for blk in range(NBLK):
                b0 = blk * BN
                ps = [pspool.tile([P, BN], mybir.dt.float32, tag="ps", name=f"ps{blk}_{m}") for m in range(MD)]
                for pos, (kind, off, nt) in enumerate(plan):
                    is8 = kind == "f8"
                    last = pos == NCH - 1
                    if is8:
                        a_c = a8pool.tile([P, max_f8_nt, BN], mybir.dt.float8e4, tag="a8")
                        src_a = a8_v[:, off : off + nt, b0 : b0 + BN]
                    else:
                        a_c = apool.tile([P, max_bf_nt, BN], mybir.dt.bfloat16, tag="a")
                        src_a = a_v[:, off : off + nt, b0 : b0 + BN]
                    if blk == 0:
                        if is8:
                            w_c = w8pool.tile([P, max_f8_nt, D], mybir.dt.float8e4, tag="w8")
                            src_w = w8_v[:, off : off + nt, :]
                        else:
                            w_c = wpool.tile([P, max_bf_nt, D], mybir.dt.bfloat16, tag="w")
                            src_w = w_v[:, off : off + nt, :]
                        w_tile_by_pos[pos] = w_c
                        if pos < SPLIT_CHUNKS:
                            # early chunks in sub-chunk DMAs: matmuls wait on
                            # a fraction of a chunk while per-DMA dispatch
                            # overhead stays amortized. Chunk-0 W rides the
                            # ACT queue so both HWDGE queues ramp in parallel.
                            wq = nc.scalar if pos <= W_ACT else nc.sync
                            prev = 0
                            for h1 in [x for x in CK0_SPLITS_A if x < nt] + [nt]:
                                nc.sync.dma_start(a_c[:, prev:h1], src_a[:, prev:h1])
                                prev = h1
                            prev = 0
                            for h1 in [x for x in CK0_SPLITS_W if x < nt] + [nt]:
                                wq.dma_start(w_c[:, prev:h1], src_w[:, prev:h1])
                                prev = h1
                        else:
                            nc.sync.dma_start(a_c[:, :nt], src_a)
                            if pos <= W_ACT:
                                nc.scalar.dma_start(w_c[:, :nt], src_w)
                            else:
                                nc.sync.dma_start(w_c[:, :nt], src_w)
                    else:
                        nc.sync.dma_start(a_c[:, :nt], src_a)
                        w_c = w_tile_by_pos[pos]
                    if blk == 0 and pos == SPLIT_CHUNKS:
                        load_bias()
                    if is8:
                        # DoubleRow: one matmul per k-tile PAIR (K=256).
                        # On the last chunk run m-outer so each subtile's
                        # accumulation finishes early and its evacuation
                        # overlaps the rest.
                        if last:
                            for m in range(MD):
                                for j in range(nt // 2):
                                    nc.tensor.matmul(
                                        ps[m][:],
                                        w_c[:, 2 * j : 2 * j + 2, ts(m, P)],
                                        a_c[:, 2 * j : 2 * j + 2],
                                        start=(pos == 0 and j == 0),
                                        stop=(j == nt // 2 - 1),
                                        perf_mode=DRMODE,
                                    )
                                evac(blk, m, ps[m])
                        else:
                            for j in range(nt // 2):
                                for m in range(MD):
                                    nc.tensor.matmul(
                                        ps[m][:],
                                        w_c[:, 2 * j : 2 * j + 2, ts(m, P)],
                                        a_c[:, 2 * j : 2 * j + 2],
                                        start=(pos == 0 and j == 0),
                                        stop=False,
                                        perf_mode=DRMODE,
                                    )
                    elif not last:
                        for k in range(nt):
                            for m in range(MD):
                                nc.tensor.matmul(
                                    ps[m][:],
                                    w_c[:, k, ts(m, P)],
                                    a_c[:, k],
                                    start=(pos == 0 and k == 0),
                                    stop=False,
                                )
                    else:
                        for m in range(MD):
                            for k in range(nt):
                                nc.tensor.matmul(
                                    ps[m][:],
                                    w_c[:, k, ts(m, P)],
                                    a_c[:, k],
                                    start=(pos == 0 and k == 0),
                                    stop=(k == nt - 1),
                                )
                            evac(blk, m, ps[m])

                if blk < NBLK - 1:
                    nc.gpsimd.collective_compute(
                        "ReduceScatter",
                        mybir.AluOpType.add,
                        replica_groups=[list(range(NCORES))],
                        ins=[partials[blk][:, :].opt()],
                        outs=[reduceds[blk][:, :].opt()],
                    )
                    nc.gpsimd.dma_start(y_ext[blk], reduceds[blk][:, :])
    nc.compile()
    return nc


def _get_nc(n_layers: int):
    if n_layers not in _nc_cache:
        _nc_cache[n_layers] = _build(n_layers)
    return _nc_cache[n_layers]


def kernel(acts: np.ndarray, W: np.ndarray, bias: np.ndarray, layer_idx) -> np.ndarray:
    global last_result
    n = int(layer_idx) + 1
    bf16 = ml_dtypes.bfloat16
    f8 = ml_dtypes.float8_e4m3
    K_LOC, K8, _ = _plan(n)

    acts32 = np.asarray(acts, dtype=np.float32)[:n]          # [n, B, F]
    W64 = np.asarray(W, dtype=np.float32)[:n] * WSCALE       # [n, F, D]
    bias = np.asarray(bias, dtype=np.float32)[:n]            # [n, D]

    nc = _get_nc(n)

    bias_t = np.ascontiguousarray(bias.T) * WSCALE  # [D, n], same on every core
    in_maps = []
    for r in range(NCORES):
        f0 = r * F_LOC
        # [n, B, F_LOC] -> [n, F_LOC, B] -> [K_loc, B]
        a_loc = np.ascontiguousarray(
            acts32[:, :, f0 : f0 + F_LOC].transpose(0, 2, 1)
        ).reshape(K_LOC, B)
        w_loc = np.ascontiguousarray(W64[:, f0 : f0 + F_LOC, :]).reshape(K_LOC, D)
        m = {
            "a_t": a_loc[K8:].astype(bf16),
            "w": w_loc[K8:].astype(bf16),
            "bias_t": bias_t,
        }
        if K8:
            m["a8"] = a_loc[:K8].astype(f8)
            m["w8"] = w_loc[:K8].astype(f8)
        in_maps.append(m)

    def run_once():
        global last_result
        last_result = run_bass_kernel_spmd(nc, in_maps, core_ids=list(range(NCORES)))
        out = np.empty((D, B), dtype=np.float32)
        last = np.zeros((D, BN), dtype=np.float32)
        for r in range(NCORES):
            y_r = np.asarray(last_result.results[r]["y"]).astype(np.float32)  # [NBLK-1, DR, BN]
            for blk in range(NBLK - 1):
                out[r * DR : (r + 1) * DR, blk * BN : (blk + 1) * BN] = y_r[blk]
            # last block: device skipped the collective; 8-way add here
            last += np.asarray(last_result.results[r]["y2"]).astype(np.float32)
        out[:, (NBLK - 1) * BN :] = last
        out *= 1.0 / WSCALE
        return np.ascontiguousarray(out.T)  # [B, D] float32

    # Guard against transient device flakes (observed: one core's contribution
    # missing from one block; first-run-after-compile stale outputs): spot-check
    # a few output elements per block against a host fp32 dot product and retry
    # the device run once on gross mismatch.
    # Threshold 0.75 clears the hybrid's quantization error (per-element err
    # std ~0.10) while still flagging a missing core-block (|diff| ~ 1.9).
    rng = np.random.default_rng(0)
    checks = [
        (int(rng.integers(blk * BN, (blk + 1) * BN)), int(m * P + rng.integers(0, P)))
        for blk in range(NBLK)
        for m in range(MD)
        for _ in range(2)
    ]
    W32 = np.asarray(W, dtype=np.float32)[:n]
    bias_sum = bias.sum(axis=0)

    def looks_good(out):
        for b_i, d_i in checks:
            ref = float(np.dot(acts32[:, b_i, :].ravel(), W32[:, :, d_i].ravel())) + float(bias_sum[d_i])
            if abs(out[b_i, d_i] - ref) > 0.75:
                return False
        return True

    out = run_once()
    if not looks_good(out):
        out = run_once()
    return out
